# revision 9
# baseline (speedup 1.0000x reference)
"""Trainium2 Bass kernel for nn_MessagePassing (vertical message passing).

Computation (per batch element b):
    y[0] = x[0]
    y[i] = x[i] + relu(conv1d_same(y[i-1], W))   for i = 1..H-1
with x (H, W, C) = (128, 256, 128) fp32, W (K, Cin, Cout) = (9, 128, 128).

Sharding: batch B=8 across the 8 NeuronCores (data parallel, no
communication); each core runs the sequential H recurrence for one batch
element. As part of sharding, kernel() lays x out per core as (H, C, W)
fp16 (the transposed layout the recurrence consumes - x only feeds the
residual add), and reassembles the output from two transposed fp16
halves to (H, W, C) fp32 during gather.

Per-core design. The serial chain is [9 conv matmuls -> relu+add ->
next step]; this schedule hides the relu+add completely:

  * state y is fp16, kept transposed (C partitions x W cols) in two
    *overlapping* tile families, 8-slot buffered over steps:
        SA = y[w -4..139]  (4 zero pad cols left)
        SB = y[w 120..259] (4 zero pad cols right)
    Each chunk's 9-tap conv reads only its own tile, so chunk A of step
    i+1 runs while chunk B's relu+add of step i is still in flight.
  * conv chunk A (out w 0..131) = 9 accumulating matmuls N=132 over SA;
    chunk B (out w 124..255) = 9 matmuls N=132 over SB. fp16 keeps full
    PE rate at N=132 (fp32r drops to 1/4 rate below N=256) and enables
    fast-weight-load; the resulting 18-LDWEIGHTS stream (~60ns each) is
    what bounds the step at ~1.08us.
  * relu+residual as 3 DVE scalar_tensor_tensor ops
        A1: SA[w 0..131]   <- max(pcA,0) + x    (right after chunk A)
        A2: SA[w 132..139] <- max(pcB,0) + x    (right after chunk B)
        B1: SB[w 124..255] <- max(pcB,0) + x
    plus an ACT-engine copy B2: SB[w 120..123] <- SA[w 120..123].
    The 12-col overlap (w 124..135) is computed by both chunks; those
    duplicated matmul columns are the price of breaking the serial
    dependency. Chunk A of step i+1 needs only {A1,A2}, chunk B only
    {B1,B2}, and each lands under PE work it does not gate.
  * no PE output transposes: rows are DMA'd directly from the state
    tiles as two fp16 DRAM tensors outA/outB [H, C, 128] (low/high w
    half, transposed). All I/O is batched 4 rows per DMA, loads on the
    SP HWDGE ring and stores split SP/ACT; 8-slot state buffering gives
    each store DMA 4 full steps to read a slot before it is rewritten.

Measured: ~132us for the 8-core batch (vs 293us baseline), max rel err
~5.8e-4 vs the fp32 reference (harness gate 2e-2).
"""

import numpy as np

B, H, W_DIM, C, K = 8, 128, 256, 128, 9
P = 128
PAD = 4
NA = 132          # chunk A conv width: out w [0, 132)
NB = 132          # chunk B conv width: out w [124, 256)
B0 = 124
SAW = 144         # SA cols: w [-4, 140)
SBW = 140         # SB cols: w [120, 260)
NSLOT = 8

_NC_CACHE = {}


def _emit_body(nc, mybir, f32, f16, x_d, oa_d, ob_d, pools, wsb, zbuf):
    (xin_pool, state_pool, pca_pool, pcb_pool) = pools
    stt = nc.vector.scalar_tensor_tensor
    mx, add = mybir.AluOpType.max, mybir.AluOpType.add

    SA8 = state_pool.tile([P, NSLOT, SAW], f16, tag="SA8", name="SA8")
    SB8 = state_pool.tile([P, NSLOT, SBW], f16, tag="SB8", name="SB8")

    x_tiles = {}

    def load_xquad(q):
        # one DMA for x rows q..q+3 (q multiple of 4)
        if q >= H:
            return
        t = xin_pool.tile([P, 4, W_DIM], f16, tag="xt", name=f"xt{q}")
        nc.sync.dma_start(t[:], x_d[q : q + 4].rearrange("r c w -> c r w"))
        x_tiles[q] = t

    def x_slice(i, c0, c1):
        return x_tiles[i - (i % 4)][:, i % 4, c0:c1]

    for q in range(0, 8, 4):
        load_xquad(q)

    # ---- prologue: pads + y_0 = x_0 ----
    for j in range(NSLOT):
        nc.vector.tensor_copy(SA8[:, j, 0:PAD], zbuf[:, 0:PAD])
        nc.vector.tensor_copy(SB8[:, j, SBW - PAD : SBW], zbuf[:, 0:PAD])
    nc.vector.tensor_copy(SA8[:, 0, PAD:SAW], x_slice(0, 0, SAW - PAD))
    nc.vector.tensor_copy(SB8[:, 0, 0 : SBW - PAD], x_slice(0, 120, W_DIM))

    def store_quad(r):
        # output rows r..r+3 (r multiple of 4): state slots r%8 .. r%8+3
        j0 = r % NSLOT
        nc.sync.dma_start(
            oa_d[r : r + 4].rearrange("r c w -> c r w"),
            SA8[:, j0 : j0 + 4, PAD : PAD + C],
        )
        nc.scalar.dma_start(
            ob_d[r : r + 4].rearrange("r c w -> c r w"),
            SB8[:, j0 : j0 + 4, 8 : 8 + C],
        )

    for i in range(1, H):
        jp, jc = (i - 1) % NSLOT, i % NSLOT

        # ---- chunk A convs (taps 5..8 need A2(i-1), cols 136+) ----
        pcA = pca_pool.tile([P, NA], f32, tag="pcA", name=f"pcA{i}")
        for k in range(K):
            nc.tensor.matmul(pcA[:], wsb[:, k, :], SA8[:, jp, k : k + NA],
                             start=(k == 0), stop=(k == K - 1))

        # DVE: A1
        stt(SA8[:, jc, PAD : PAD + NA], pcA[:], 0.0, x_slice(i, 0, NA),
            op0=mx, op1=add)
        # ACT: B2 = copy of A1's w 120..123 into SB's left halo
        nc.scalar.copy(SB8[:, jc, 0:PAD], SA8[:, jc, 124:128])

        # ---- chunk B convs ----
        pcB = pcb_pool.tile([P, NB], f32, tag="pcB", name=f"pcB{i}")
        for k in range(K):
            nc.tensor.matmul(pcB[:], wsb[:, k, :], SB8[:, jp, k : k + NB],
                             start=(k == 0), stop=(k == K - 1))

        # DVE: A2 then B1
        stt(SA8[:, jc, PAD + NA : SAW], pcB[:, 8:16], 0.0,
            x_slice(i, NA, NA + 8), op0=mx, op1=add)
        stt(SB8[:, jc, PAD : PAD + NB], pcB[:], 0.0, x_slice(i, B0, W_DIM),
            op0=mx, op1=add)

        # output rows (i-4 .. i-1) once final; their slots idle 4 more steps
        if i % 4 == 0 and i >= 4:
            store_quad(i - 4)

        if i % 4 == 3:
            load_xquad(i + 5)
            x_tiles.pop(i - 7, None)

    # epilogue: rows 124..127
    store_quad(H - 4)


def _build_nc(reps=1):
    import contextlib

    import concourse.tile as tile
    from concourse import bacc, mybir

    f32 = mybir.dt.float32
    f16 = mybir.dt.float16

    nc = bacc.Bacc("TRN2", target_bir_lowering=False, debug=False, num_devices=B)
    x_d = nc.dram_tensor("x", [H, C, W_DIM], f16, kind="ExternalInput").ap()
    w_d = nc.dram_tensor("w", [K, C, C], f32, kind="ExternalInput").ap()
    oa_d = nc.dram_tensor("outA", [H, C, C], f16, kind="ExternalOutput").ap()
    ob_d = nc.dram_tensor("outB", [H, C, C], f16, kind="ExternalOutput").ap()

    with tile.TileContext(nc) as tc:
        with (
            tc.tile_pool(name="xin", bufs=4) as xin_pool,
            tc.tile_pool(name="state", bufs=1) as state_pool,
            tc.tile_pool(name="const", bufs=1) as const_pool,
            tc.tile_pool(name="pca", bufs=3, space="PSUM") as pca_pool,
            tc.tile_pool(name="pcb", bufs=3, space="PSUM") as pcb_pool,
        ):
            wsb_raw = const_pool.tile([P, K, C], f32, name="wsb_raw")
            nc.sync.dma_start(wsb_raw[:], w_d.rearrange("k ci co -> ci k co"))
            wsb = const_pool.tile([P, K, C], f16, name="wsb")
            nc.vector.tensor_copy(wsb[:], wsb_raw[:])

            zbuf = const_pool.tile([P, PAD], f32, name="zbuf")
            nc.vector.memset(zbuf[:], 0.0)

            pools = (xin_pool, state_pool, pca_pool, pcb_pool)
            rep_ctx = tc.For_i(0, reps, 1) if reps > 1 else contextlib.nullcontext()
            with rep_ctx:
                _emit_body(nc, mybir, f32, f16, x_d, oa_d, ob_d, pools, wsb,
                           zbuf)

    nc.compile()
    return nc


def _get_nc():
    if "nc" not in _NC_CACHE:
        _NC_CACHE["nc"] = _build_nc()
    return _NC_CACHE["nc"]


def make_in_maps(x, W):
    x = np.asarray(x, dtype=np.float32)
    W = np.asarray(W, dtype=np.float32)
    return [
        {
            "x": np.ascontiguousarray(
                x[b].transpose(0, 2, 1).astype(np.float16)
            ),
            "w": W,
        }
        for b in range(B)
    ]


def assemble_out(res_map):
    oa = np.asarray(res_map["outA"])  # (H, C, 128) fp16, w 0..127
    ob = np.asarray(res_map["outB"])  # (H, C, 128) fp16, w 128..255
    return np.concatenate(
        [oa.transpose(0, 2, 1), ob.transpose(0, 2, 1)], axis=1
    ).astype(np.float32)


def kernel(x, W):
    from concourse.bass_utils import run_bass_kernel_spmd

    x = np.asarray(x, dtype=np.float32)
    W = np.asarray(W, dtype=np.float32)
    nc = _get_nc()
    res = run_bass_kernel_spmd(nc, make_in_maps(x, W), core_ids=list(range(B)))
    return np.stack([assemble_out(res.results[b]) for b in range(B)], axis=0)


def assemble_sharded(outs):
    """outs: tuple of per-name arrays concatenated over cores (test.py's
    sharded-jit output). Returns (B, H, W, C) fp32."""
    oa = np.asarray(outs[0]).reshape(B, H, C, C)
    ob = np.asarray(outs[1]).reshape(B, H, C, C)
    return np.stack(
        [assemble_out({"outA": oa[b], "outB": ob[b]}) for b in range(B)]
    )


# revision 10
# speedup vs baseline: 1.1067x; 1.1067x over previous
"""Trainium2 Bass kernel for nn_MessagePassing (vertical message passing).

Computation (per batch element b):
    y[0] = x[0]
    y[i] = x[i] + relu(conv1d_same(y[i-1], W))   for i = 1..H-1
with x (H, W, C) = (128, 256, 128) fp32, W (K, Cin, Cout) = (9, 128, 128).

Sharding: batch B=8 across the 8 NeuronCores (data parallel, no
communication); each core runs the sequential H recurrence for one batch
element. As part of sharding, kernel() lays x out per core as (H, C, W)
fp16 (the transposed layout the recurrence consumes - x only feeds the
residual add), and reassembles the output from two transposed fp16
halves to (H, W, C) fp32 during gather.

Per-core design. The serial chain is [9 conv matmuls -> relu+add ->
next step]; this schedule hides the relu+add completely:

  * state y is fp16, kept transposed (C partitions x W cols) in two
    *overlapping* tile families, 8-slot buffered over steps:
        SA = y[w -4..139]  (4 zero pad cols left)
        SB = y[w 120..259] (4 zero pad cols right)
    Each chunk's 9-tap conv reads only its own tile, so chunk A of step
    i+1 runs while chunk B's relu+add of step i is still in flight.
  * conv chunk A (out w 0..131) = 9 accumulating matmuls N=132 over SA;
    chunk B (out w 124..255) = 9 matmuls N=132 over SB. fp16 keeps full
    PE rate at N=132 (fp32r drops to 1/4 rate below N=256) and enables
    fast-weight-load; the resulting 18-LDWEIGHTS stream (~60ns each) is
    what bounds the step at ~1.08us.
  * relu+residual as 3 DVE scalar_tensor_tensor ops
        A1: SA[w 0..131]   <- max(pcA,0) + x    (right after chunk A)
        A2: SA[w 132..139] <- max(pcB,0) + x    (right after chunk B)
        B1: SB[w 124..255] <- max(pcB,0) + x
    plus an ACT-engine copy B2: SB[w 120..123] <- SA[w 120..123].
    The 12-col overlap (w 124..135) is computed by both chunks; those
    duplicated matmul columns are the price of breaking the serial
    dependency. Chunk A of step i+1 needs only {A1,A2}, chunk B only
    {B1,B2}, and each lands under PE work it does not gate.
  * no PE output transposes: rows are DMA'd directly from the state
    tiles as two fp16 DRAM tensors outA/outB [H, C, 128] (low/high w
    half, transposed). All I/O is batched 4 rows per DMA, loads on the
    SP HWDGE ring and stores split SP/ACT; 8-slot state buffering gives
    each store DMA 4 full steps to read a slot before it is rewritten.

Measured: ~132us for the 8-core batch (vs 293us baseline), max rel err
~5.8e-4 vs the fp32 reference (harness gate 2e-2).
"""

import numpy as np

B, H, W_DIM, C, K = 8, 128, 256, 128, 9
P = 128
PAD = 4
NA = 132          # chunk A conv width: out w [0, 132)
NB = 132          # chunk B conv width: out w [124, 256)
B0 = 124
SAW = 144         # SA cols: w [-4, 140)
SBW = 140         # SB cols: w [120, 260)
NSLOT = 8

_NC_CACHE = {}


def _emit_body(nc, mybir, f32, f16, x_d, oa_d, ob_d, pools, wsb, zbuf):
    (xin_pool, state_pool, pca_pool, pcb_pool) = pools
    stt = nc.vector.scalar_tensor_tensor
    mx, add = mybir.AluOpType.max, mybir.AluOpType.add

    SA8 = state_pool.tile([P, NSLOT, SAW], f16, tag="SA8", name="SA8")
    SB8 = state_pool.tile([P, NSLOT, SBW], f16, tag="SB8", name="SB8")

    x_tiles = {}

    def load_xquad(q):
        # one DMA for x rows q..q+3 (q multiple of 4)
        if q >= H:
            return
        t = xin_pool.tile([P, 4, W_DIM], f16, tag="xt", name=f"xt{q}")
        nc.sync.dma_start(t[:], x_d[q : q + 4].rearrange("r c w -> c r w"))
        x_tiles[q] = t

    def x_slice(i, c0, c1):
        return x_tiles[i - (i % 4)][:, i % 4, c0:c1]

    for q in range(0, 8, 4):
        load_xquad(q)

    # ---- prologue: pads + y_0 = x_0 ----
    for j in range(NSLOT):
        nc.vector.tensor_copy(SA8[:, j, 0:PAD], zbuf[:, 0:PAD])
        nc.vector.tensor_copy(SB8[:, j, SBW - PAD : SBW], zbuf[:, 0:PAD])
    nc.vector.tensor_copy(SA8[:, 0, PAD:SAW], x_slice(0, 0, SAW - PAD))
    nc.vector.tensor_copy(SB8[:, 0, 0 : SBW - PAD], x_slice(0, 120, W_DIM))

    def store_quad(r):
        # output rows r..r+3 (r multiple of 4): state slots r%8 .. r%8+3
        j0 = r % NSLOT
        nc.sync.dma_start(
            oa_d[r : r + 4].rearrange("r c w -> c r w"),
            SA8[:, j0 : j0 + 4, PAD : PAD + C],
        )
        nc.scalar.dma_start(
            ob_d[r : r + 4].rearrange("r c w -> c r w"),
            SB8[:, j0 : j0 + 4, 8 : 8 + C],
        )

    for i in range(1, H):
        jp, jc = (i - 1) % NSLOT, i % NSLOT

        # ---- chunk A convs (taps 5..8 need A2(i-1), cols 136+) ----
        pcA = pca_pool.tile([P, NA], f32, tag="pcA", name=f"pcA{i}")
        for k in range(K):
            nc.tensor.matmul(pcA[:], wsb[:, k, :], SA8[:, jp, k : k + NA],
                             start=(k == 0), stop=(k == K - 1))

        # DVE: A1
        stt(SA8[:, jc, PAD : PAD + NA], pcA[:], 0.0, x_slice(i, 0, NA),
            op0=mx, op1=add)
        # DVE: B2 = copy of A1's w 120..123 into SB's left halo (kept off
        # ACT, whose store-descriptor generation would delay it behind a
        # ~650ns DMA-gen every 4th step and stall the next B phase)
        nc.vector.tensor_copy(SB8[:, jc, 0:PAD], SA8[:, jc, 124:128])

        # ---- chunk B convs ----
        pcB = pcb_pool.tile([P, NB], f32, tag="pcB", name=f"pcB{i}")
        for k in range(K):
            nc.tensor.matmul(pcB[:], wsb[:, k, :], SB8[:, jp, k : k + NB],
                             start=(k == 0), stop=(k == K - 1))

        # DVE: A2 then B1
        stt(SA8[:, jc, PAD + NA : SAW], pcB[:, 8:16], 0.0,
            x_slice(i, NA, NA + 8), op0=mx, op1=add)
        stt(SB8[:, jc, PAD : PAD + NB], pcB[:], 0.0, x_slice(i, B0, W_DIM),
            op0=mx, op1=add)

        # output rows (i-4 .. i-1) once final; their slots idle 4 more steps
        if i % 4 == 0 and i >= 4:
            store_quad(i - 4)

        if i % 4 == 3:
            load_xquad(i + 5)
            x_tiles.pop(i - 7, None)

    # epilogue: rows 124..127
    store_quad(H - 4)


def _build_nc(reps=1):
    import contextlib

    import concourse.tile as tile
    from concourse import bacc, mybir

    f32 = mybir.dt.float32
    f16 = mybir.dt.float16

    nc = bacc.Bacc("TRN2", target_bir_lowering=False, debug=False, num_devices=B)
    x_d = nc.dram_tensor("x", [H, C, W_DIM], f16, kind="ExternalInput").ap()
    w_d = nc.dram_tensor("w", [K, C, C], f32, kind="ExternalInput").ap()
    oa_d = nc.dram_tensor("outA", [H, C, C], f16, kind="ExternalOutput").ap()
    ob_d = nc.dram_tensor("outB", [H, C, C], f16, kind="ExternalOutput").ap()

    with tile.TileContext(nc) as tc:
        with (
            tc.tile_pool(name="xin", bufs=4) as xin_pool,
            tc.tile_pool(name="state", bufs=1) as state_pool,
            tc.tile_pool(name="const", bufs=1) as const_pool,
            tc.tile_pool(name="pca", bufs=3, space="PSUM") as pca_pool,
            tc.tile_pool(name="pcb", bufs=3, space="PSUM") as pcb_pool,
        ):
            wsb_raw = const_pool.tile([P, K, C], f32, name="wsb_raw")
            nc.sync.dma_start(wsb_raw[:], w_d.rearrange("k ci co -> ci k co"))
            wsb = const_pool.tile([P, K, C], f16, name="wsb")
            nc.vector.tensor_copy(wsb[:], wsb_raw[:])

            zbuf = const_pool.tile([P, PAD], f32, name="zbuf")
            nc.vector.memset(zbuf[:], 0.0)

            pools = (xin_pool, state_pool, pca_pool, pcb_pool)
            rep_ctx = tc.For_i(0, reps, 1) if reps > 1 else contextlib.nullcontext()
            with rep_ctx:
                _emit_body(nc, mybir, f32, f16, x_d, oa_d, ob_d, pools, wsb,
                           zbuf)

    nc.compile()
    return nc


def _get_nc():
    if "nc" not in _NC_CACHE:
        _NC_CACHE["nc"] = _build_nc()
    return _NC_CACHE["nc"]


def make_in_maps(x, W):
    x = np.asarray(x, dtype=np.float32)
    W = np.asarray(W, dtype=np.float32)
    return [
        {
            "x": np.ascontiguousarray(
                x[b].transpose(0, 2, 1).astype(np.float16)
            ),
            "w": W,
        }
        for b in range(B)
    ]


def assemble_out(res_map):
    oa = np.asarray(res_map["outA"])  # (H, C, 128) fp16, w 0..127
    ob = np.asarray(res_map["outB"])  # (H, C, 128) fp16, w 128..255
    return np.concatenate(
        [oa.transpose(0, 2, 1), ob.transpose(0, 2, 1)], axis=1
    ).astype(np.float32)


def kernel(x, W):
    from concourse.bass_utils import run_bass_kernel_spmd

    x = np.asarray(x, dtype=np.float32)
    W = np.asarray(W, dtype=np.float32)
    nc = _get_nc()
    res = run_bass_kernel_spmd(nc, make_in_maps(x, W), core_ids=list(range(B)))
    return np.stack([assemble_out(res.results[b]) for b in range(B)], axis=0)


def assemble_sharded(outs):
    """outs: tuple of per-name arrays concatenated over cores (test.py's
    sharded-jit output). Returns (B, H, W, C) fp32."""
    oa = np.asarray(outs[0]).reshape(B, H, C, C)
    ob = np.asarray(outs[1]).reshape(B, H, C, C)
    return np.stack(
        [assemble_out({"outA": oa[b], "outB": ob[b]}) for b in range(B)]
    )
